# revision 1
# baseline (speedup 1.0000x reference)
"""Linear-chain CRF partition function (forward algorithm) on 8 Trainium2 cores.

Math: the reference scan is, per batch b,
    alpha_{t+1}[j] = logit[b,t,j] + logsumexp_k(trans[j,k] + alpha_t[k])
masked so rows stop updating at t = lens[b], then
    norm[b] = logsumexp_j(alpha_{lens[b]}[j] + trans[stop, j]).

Substituting p_t = exp(alpha_t - C*(t+1)) turns the whole scan into LINEAR
space with a constant per-step drift C (no per-step exp/log/max needed):
    s_t = W^T p_t          (W[k,j] = exp(trans_aug[j,k]), one matmul)
    p_{t+1} = s_t  * q_t    (q_t[j,b] = exp(logit[b,t,j] - C), elementwise)
W gets an extra output row j=102 carrying exp(trans[stop, k]) so that
    s_t[102, b] = exp(logsumexp_j(trans[stop,j] + alpha_t[j]) - C*(t+1))
i.e. row 102 of each step's matmul IS the partition function readout for
sequences with lens[b] == t.  No masking is needed on-device: dead rows keep
evolving harmlessly and are simply never read after their readout step.

With C = 5.6103 (mean drift of alpha for this input distribution) the scaled
state stays within e^{+-13} for all 512 steps => comfortably inside fp32
range, and the fp32 linear recurrence matches a fp64 oracle to ~1e-7 rel.

Sharding: data-parallel over batch. 512 batches / 8 cores = 64 columns per
core; transitions replicated. The scan over T=512 is sequential per core,
batch columns ride the matmul/DVE free dimension.
"""

import numpy as np

import concourse.bacc as bacc
import concourse.mybir as mybir
import concourse.tile as tile
from concourse.bass_utils import run_bass_kernel_spmd

# Problem shape (hardcoded per contract: kernel.py must be self-contained).
B_TOTAL = 512
T = 512
L = 102          # labels incl. start/stop
LP = L + 1       # + readout row (stop-transition dot product)
NCORES = 8
B = B_TOTAL // NCORES   # batch columns per core
START = L - 2
STOP = L - 1
C = np.float32(5.6103331)  # per-step drift stabilizer

# Tunables (cost-model-swept; final config: 168.1us — loop at 100% of the
# DVE busy floor of 1024 multiplies x 158ns, startup 2.9us, drain 3.4us)
P_PIPES = 2              # independent column pipelines (PE/DVE overlap)
WP = B // P_PIPES        # columns per pipeline
R_SLOTS = 64             # circular p-store depth (per pipeline), multiple of 2*RD_CHUNK
RD_CHUNK = 32            # readout DMA batching (slots per DMA)
Q_CHUNKS = 64            # q preload DMA chunks

FP32 = mybir.dt.float32


def _build_nc(steps: int = T):
    nc = bacc.Bacc()
    qs = nc.dram_tensor("qs", [LP, steps * B], FP32, kind="ExternalInput")
    # w ([L, LP] transition weights) and p0 ([L, B] initial state) share one
    # tensor/DMA so every matmul carries a single sem wait (walrus's
    # LDWEIGHTS descriptor rejects >1 sync wait on a Matmult).
    wp = nc.dram_tensor("wp", [L, LP + B], FP32, kind="ExternalInput")
    rb = nc.dram_tensor("rb", [steps, B], FP32, kind="ExternalOutput")

    with tile.TileContext(nc) as tc:
        with (
            tc.tile_pool(name="const", bufs=1) as cpool,
            tc.tile_pool(name="qpool", bufs=1) as qpool,
            tc.tile_pool(name="ppool", bufs=1) as ppool,
            tc.tile_pool(name="psum", bufs=8, space="PSUM") as psum_pool,
        ):
            wpt = cpool.tile([L, LP + B], FP32)
            # ACT's HWDGE queue: overlaps with the q lead chunk on SP's queue
            # so the first matmul isn't serialized behind both transfers.
            nc.scalar.dma_start(wpt[:], wp[:])
            wt = wpt[:, :LP]
            p0t = wpt[:, LP:]

            qst = qpool.tile([LP, steps * B], FP32)
            # Non-uniform chunking: tiny leading chunks so the first TT isn't
            # gated on a large transfer, then uniform large chunks.
            n_chunks = min(Q_CHUNKS, steps)
            qcols = steps * B // n_chunks
            lead = [B * 2] if qcols > B * 2 else []
            pos = 0
            for c in lead + [qcols - sum(lead)] + [qcols] * (n_chunks - 1):
                nc.sync.dma_start(qst[:, pos:pos + c], qs[:, pos:pos + c])
                pos += c
            assert pos == steps * B

            # circular per-pipeline state stores; slot k writes cols
            # [(k%R)*WP, (k%R+1)*WP); row LP-1 is the readout row.
            pst = [
                ppool.tile([LP, R_SLOTS * WP], FP32, name=f"pst{g}")
                for g in range(P_PIPES)
            ]

            for k in range(steps):
                for g in range(P_PIPES):
                    ps = psum_pool.tile([LP, WP], FP32)
                    if k == 0:
                        rhs = p0t[:, g * WP:(g + 1) * WP]
                    else:
                        c0 = ((k - 1) % R_SLOTS) * WP
                        rhs = pst[g][0:L, c0:c0 + WP]
                    nc.tensor.matmul(ps[:], wt, rhs)
                    c1 = (k % R_SLOTS) * WP
                    qc = k * B + g * WP
                    nc.vector.tensor_mul(
                        pst[g][:, c1:c1 + WP], ps[:], qst[:, qc:qc + WP]
                    )
                if k % RD_CHUNK == RD_CHUNK - 1:
                    k0 = k - (RD_CHUNK - 1)
                    c0 = (k0 % R_SLOTS) * WP
                    for g in range(P_PIPES):
                        src = pst[g][LP - 1:LP, c0:c0 + RD_CHUNK * WP]
                        # ACT's HWDGE queues, so readouts never sit behind the
                        # huge q-preload transfers on SP's queues (a readout
                        # stuck there stalls the circular pstore rewrite).
                        nc.scalar.dma_start(
                            rb[k0:k0 + RD_CHUNK, g * WP:(g + 1) * WP], src
                        )
    nc.finalize()
    return nc


def _host_prep(logits: np.ndarray, transitions: np.ndarray):
    """Build per-core device inputs."""
    q = np.exp(logits.astype(np.float32) - C)  # [B_TOTAL, T, L]
    qt = np.transpose(q, (2, 1, 0))            # [L, T, B_TOTAL]
    trans_aug = np.concatenate(
        [transitions, transitions[STOP:STOP + 1]], axis=0
    ).astype(np.float32)                       # [LP, L]
    wp = np.zeros((L, LP + B), np.float32)
    wp[:, :LP] = np.exp(trans_aug).T           # [L, LP]
    wp[START, LP:] = np.exp(-C)                # p0
    in_maps = []
    for c in range(NCORES):
        qs_c = np.empty((LP, T * B), np.float32)
        qs_c[:L] = qt[:, :, c * B:(c + 1) * B].reshape(L, T * B)
        qs_c[L] = 1.0
        in_maps.append({"qs": qs_c, "wp": wp})
    return in_maps


def kernel(logits: np.ndarray, transitions: np.ndarray, lens: np.ndarray) -> np.ndarray:
    assert logits.shape == (B_TOTAL, T, L)
    in_maps = _host_prep(np.asarray(logits), np.asarray(transitions))
    nc = _build_nc()
    res = run_bass_kernel_spmd(nc, in_maps, list(range(NCORES))).results
    rb_full = np.concatenate([r["rb"] for r in res], axis=1)  # [T, B_TOTAL]
    lens = np.asarray(lens).astype(np.int64)
    vals = rb_full[lens, np.arange(B_TOTAL)]
    norm = np.log(vals) + C * (lens.astype(np.float32) + 1.0)
    return norm.astype(np.float32)

